# revision 2
# baseline (speedup 1.0000x reference)
"""Multi-head self-attention (B=16, N=784, D=768, H=12) on 8 trn2 cores.

Sharding: pure data-parallel over batch (2 batches per core, no collectives).
All matmuls bf16 with fp32 PSUM accumulation.

v2 over baseline: score matmuls for a head pair (which share one qkt tile —
head 2p at partitions 0:64, head 2p+1 at 64:128) are emitted back-to-back
into two separate PSUM tiles. They use disjoint PE row groups
(tile_position (0,0)/(64,0)) so the hardware runs them concurrently,
doubling effective PE-array utilization for the scores phase (K=64 would
otherwise use half the array). PV keeps the ones-augmented V slab (M=65,
softmax denominator in row 64); PV for the previous pair is fed from a work
queue, a few matmuls per tj slot, using one [65, 1024] PSUM tile at a time.
NOTE: col-tiled concurrent matmul pairs (two PE writers into one PSUM bank)
fault the device — do not re-introduce them.

PSUM budget (8 banks): scores ring 2 x [128,1024] (4 banks) + aux ring
1 x [128,1024] (2, projection accumulators, emitted atomically per chunk)
+ PV accumulator 1 x [65,1024] (2).

Loop order everywhere keeps the col-group innermost so consecutive MMs share
the stationary operand (saves the PE weight-swap drain bubble).
"""

from collections import deque
from contextlib import ExitStack

import ml_dtypes
import numpy as np

import concourse.mybir as mybir
import concourse.tile as tile
from concourse import bacc
from concourse.bass_utils import run_bass_kernel_spmd

dt = mybir.dt
AF = mybir.ActivationFunctionType

B, N, D = 16, 784, 768
H, HD = 12, 64
F3 = 3 * D  # 2304
N_CORES = 8
BPC = B // N_CORES  # batches per core

T_CHUNKS = [(i * 128, min(128, N - i * 128)) for i in range((N + 127) // 128)]
NT = len(T_CHUNKS)  # 7
ND = D // 128  # 6
COLS_N = [(0, 512), (512, N - 512)]
COLS_D = [(0, 512), (512, D - 512)]

BF = dt.bfloat16


def _setup_consts(nc, P, aps):
    bqc = P["konst"].tile([128, F3 // 128], dt.float32, name="bqc")
    nc.sync.dma_start(bqc[:], aps["bqc"][:])
    bqv_r = P["konst"].tile([1, D], dt.float32, name="bqv_r")
    nc.sync.dma_start(bqv_r[:], aps["bqv"][:])
    bqv_bc = P["konst"].tile([128, D], dt.float32, name="bqv_bc")
    nc.gpsimd.partition_broadcast(bqv_bc[:], bqv_r[0:1, :])
    bo_r = P["konst"].tile([1, D], dt.float32, name="bo_r")
    nc.sync.dma_start(bo_r[:], aps["bo"][:])
    bo_bc = P["konst"].tile([128, D], dt.float32, name="bo_bc")
    nc.gpsimd.partition_broadcast(bo_bc[:], bo_r[0:1, :])

    wq16, wo16 = [], []
    for di in range(ND):
        w = P["wq"].tile([128, F3], BF, name=f"wq{di}", tag="wq")
        nc.sync.dma_start(w[:, 2 * D:F3], aps["wqkv"][di * 128:(di + 1) * 128, 2 * D:F3])
        wq16.append(w)
    for di in range(ND):
        w = wq16[di]
        eng = nc.sync if di % 2 == 0 else nc.scalar
        eng.dma_start(w[:, 0:2 * D], aps["wqkv"][di * 128:(di + 1) * 128, 0:2 * D])
    for di in range(ND):
        w = P["wo"].tile([128, D], BF, name=f"wo{di}", tag="wo")
        nc.sync.dma_start(w[:], aps["wo"][di * 128:(di + 1) * 128, :])
        wo16.append(w)
    return dict(bqc=bqc, bqv_bc=bqv_bc, bo_bc=bo_bc, wq16=wq16, wo16=wo16)


def _gen_a(nc, P, C, aps, b, st):
    xt16 = [P["xt"].tile([128, N], BF, name=f"xt{b}_{di}", tag="xt")
            for di in range(ND)]
    for di in range(ND):
        nc.gpsimd.dma_start(xt16[di][:], aps["xs"][b, di * 128:(di + 1) * 128, :])
    yield
    st[f"xt{b}"] = xt16


def _gen_b_qk(nc, P, C, b, st):
    """Q,K transposed layout: 12 tiles [128, 784]. di outer / cg inner."""
    xt16 = st[f"xt{b}"]
    qkt16 = [None] * 12
    st[f"qkt{b}"] = qkt16
    for fi in [0, 6, 1, 7, 2, 8, 3, 9, 4, 10, 5, 11]:
        qk_ps = P["ps_aux"].tile([128, 1024], dt.float32, name="qk_ps", tag="aux")
        for di in range(ND):
            for (c0, cw) in COLS_N:
                nc.tensor.matmul(
                    qk_ps[:, c0:c0 + cw],
                    C["wq16"][di][:, fi * 128:(fi + 1) * 128],
                    xt16[di][:, c0:c0 + cw],
                    start=(di == 0), stop=(di == ND - 1))
        q = P["qkt"].tile([128, N], BF, name=f"qkt{b}_{fi}", tag="qkt")
        nc.vector.tensor_scalar_add(q[:], qk_ps[:, 0:N], C["bqc"][0:128, fi:fi + 1])
        qkt16[fi] = q
        yield


def _gen_b_v(nc, P, C, b, st):
    """V slab [t, 12, 65]: 65th column is 1.0 (softmax denominator trick)."""
    xt16 = st[f"xt{b}"]
    vt16 = []
    st[f"vt{b}"] = vt16
    for (t0, p), ti in zip(T_CHUNKS, range(NT)):
        v_ps = P["ps_aux"].tile([128, 1024], dt.float32, name="v_ps", tag="aux")
        for di in range(ND):
            for (c0, cw) in COLS_D:
                nc.tensor.matmul(
                    v_ps[0:p, c0:c0 + cw],
                    xt16[di][:, t0:t0 + p],
                    C["wq16"][di][:, 2 * D + c0:2 * D + c0 + cw],
                    start=(di == 0), stop=(di == ND - 1))
        vt = P["vt"].tile([128, H, HD + 1], BF, name=f"vt{b}_{ti}", tag="vt")
        nc.vector.tensor_tensor(
            vt[0:p, :, 0:HD],
            v_ps[0:p, 0:D].rearrange("p (h d) -> p h d", h=H),
            C["bqv_bc"][0:p, 0:D].rearrange("p (h d) -> p h d", h=H),
            mybir.AluOpType.add)
        nc.vector.memset(vt[0:p, :, HD:HD + 1], 1.0)
        vt16.append(vt)
        yield


def _pv_thunks(nc, P, b, st, p):
    """Work queue for pair p's PV + normalize (two heads, sequential).

    Per head: two PSUM accumulators (one per col group, 1 bank each), 14
    accumulating MMs tj-outer / cg-inner (consecutive MMs share the vt
    stationary; ex consumption is tj-monotone), then drain + normalize
    into ot16[p] rows hh*64. Row 64 of the accumulators is the softmax
    denominator (ones column of the V slab).
    """
    vt16, ex = st[f"vt{b}"], st[f"ex{b}_{p}"]
    work = deque()
    holder = {}

    def _alloc(hh):
        def f():
            holder[0] = P["ps_ot"].tile([HD + 1, 512], dt.float32,
                                        name=f"pv0_{p}_{hh}", tag="ot0")
            holder[1] = P["ps_ot"].tile([HD + 1, COLS_N[1][1]], dt.float32,
                                        name=f"pv1_{p}_{hh}", tag="ot1")
        return f

    def _mms(hh, tjs):
        h = 2 * p + hh

        def f():
            for tj in tjs:
                (t0, pj) = T_CHUNKS[tj]
                for ci, (c0, cw) in enumerate(COLS_N):
                    nc.tensor.matmul(
                        holder[ci][0:HD + 1, 0:cw],
                        vt16[tj][0:pj, h, 0:HD + 1],
                        ex[2 * tj + hh][0:pj, c0:c0 + cw],
                        start=(tj == 0), stop=(tj == NT - 1))
        return f

    def _finish(hh):
        def f():
            if hh == 0:
                ot = P["ot"].tile([128, N], BF, name=f"ot{b}_{p}", tag="ot")
                st[f"ot{b}"][p] = ot
            ot = st[f"ot{b}"][p]
            osb = P["osb"].tile([HD + 1, N], dt.float32, name="osb65", tag="osb")
            for ci, (c0, cw) in enumerate(COLS_N):
                nc.vector.tensor_copy(osb[0:HD + 1, c0:c0 + cw],
                                      holder[ci][0:HD + 1, 0:cw])
            srow = P["recp"].tile([1, N], dt.float32, name="srow", tag="srow")
            nc.vector.tensor_copy(srow[0:1, :], osb[HD:HD + 1, :])
            rec = P["recp"].tile([1, N], dt.float32, name="rec", tag="rec")
            nc.vector.reciprocal_approx_fast(rec[0:1, :], srow[0:1, :])
            brec = P["brec"].tile([HD, N], dt.float32, name="brec", tag="brec")
            nc.gpsimd.partition_broadcast(brec[0:HD, :], rec[0:1, :])
            ro = hh * HD
            nc.vector.tensor_mul(ot[ro:ro + HD, :], osb[0:HD, :], brec[0:HD, :])
        return f

    for hh in range(2):
        work.append(_alloc(hh))
        work.append(_mms(hh, range(0, 2)))
        work.append(_mms(hh, range(2, 4)))
        work.append(_mms(hh, range(4, 6)))
        work.append(_mms(hh, range(6, NT)))
        work.append(_finish(hh))
    return work


def _pop(work, k):
    n = 0
    while work and n < k:
        work.popleft()()
        n += 1
    return n


def _gen_c(nc, P, C, b, st):
    """Attention for batch b. Yields once per (pair, tj)."""
    qkt16 = st[f"qkt{b}"]
    st[f"ot{b}"] = [None] * 6
    pv_work = deque()
    for p in range(6):
        qt, kt = qkt16[p], qkt16[6 + p]
        ex = [P["ex"].tile([128, N], BF, name="ex", tag="ex") for _ in range(2 * NT)]
        st[f"ex{b}_{p}"] = ex
        if p >= 1:
            assert not pv_work
            pv_work = _pv_thunks(nc, P, b, st, p - 1)
        for (t0, pj), tj in zip(T_CHUNKS, range(NT)):
            # prev-pair PV (and the outer loop's filler, via yield) go in
            # front of the scores so the PE reaches the next score pair
            # after ACT has drained the previous one
            _pop(pv_work, 3)
            yield
            sc_a = P["ps_sc"].tile([128, 1024], dt.float32, name="sc_a", tag="sc")
            sc_b = P["ps_sc"].tile([128, 1024], dt.float32, name="sc_b", tag="sc")
            for hh, sc in ((0, sc_a), (1, sc_b)):
                ro = hh * HD
                for (c0, cw) in COLS_N:
                    nc.tensor.matmul(
                        sc[0:pj, c0:c0 + cw],
                        kt[ro:ro + HD, t0:t0 + pj],
                        qt[ro:ro + HD, c0:c0 + cw],
                        start=True, stop=True)
            for hh, sc in ((0, sc_a), (1, sc_b)):
                nc.scalar.activation(ex[2 * tj + hh][0:pj, :], sc[0:pj, 0:N],
                                     AF.Exp, scale=float(HD) ** -0.5)
        _pop(pv_work, 99)  # finish prev pair's PV before ex tiles rotate far
    # epilogue: last pair's PV
    pv_work = _pv_thunks(nc, P, b, st, 5)
    _pop(pv_work, 999)


def _gen_d(nc, P, C, aps, b, st, pool="ps_aux"):
    ot16 = st[f"ot{b}"]
    for (t0, p), ti in zip(T_CHUNKS, range(NT)):
        y_ps = P[pool].tile([128, 1024], dt.float32, name="y_ps",
                            tag="sc" if pool == "ps_sc" else "aux")
        for oi in range(ND):
            for (c0, cw) in COLS_D:
                nc.tensor.matmul(
                    y_ps[0:p, c0:c0 + cw],
                    ot16[oi][:, t0:t0 + p],
                    C["wo16"][oi][:, c0:c0 + cw],
                    start=(oi == 0), stop=(oi == ND - 1))
        y32 = P["yout"].tile([128, D], dt.float32, name="y32", tag="y32")
        nc.vector.tensor_add(y32[0:p, :], y_ps[0:p, 0:D], C["bo_bc"][0:p, :])
        nc.gpsimd.dma_start(aps["ys"][b, t0:t0 + p, :], y32[0:p, :])
        yield


def _exhaust(g):
    for _ in g:
        pass


def _pull(g, k):
    n = 0
    for _ in range(k):
        try:
            next(g)
        except StopIteration:
            break
        n += 1
    return n


POOL_SPECS = [
    ("konst", 1, "SBUF"), ("wq", ND, "SBUF"), ("wo", ND, "SBUF"),
    ("xt", 2 * ND, "SBUF"),
    ("qkt", 24, "SBUF"), ("vt", 2 * NT, "SBUF"), ("ex", 18, "SBUF"),
    ("ot", 12, "SBUF"), ("osb", 3, "SBUF"), ("recp", 2, "SBUF"),
    ("brec", 2, "SBUF"),
    ("yout", 2, "SBUF"),
    ("ps_sc", 2, "PSUM"), ("ps_aux", 1, "PSUM"), ("ps_ot", 1, "PSUM"),
]


def build():
    nc = bacc.Bacc("TRN2", target_bir_lowering=False, debug=False)

    aps = {
        "xs": nc.dram_tensor("xs", [BPC, D, N], BF, kind="ExternalInput").ap(),
        "wqkv": nc.dram_tensor("wqkv", [D, F3], BF, kind="ExternalInput").ap(),
        "bqc": nc.dram_tensor("bqc", [128, F3 // 128], dt.float32, kind="ExternalInput").ap(),
        "bqv": nc.dram_tensor("bqv", [1, D], dt.float32, kind="ExternalInput").ap(),
        "wo": nc.dram_tensor("wo", [D, D], BF, kind="ExternalInput").ap(),
        "bo": nc.dram_tensor("bo", [1, D], dt.float32, kind="ExternalInput").ap(),
        "ys": nc.dram_tensor("ys", [BPC, N, D], dt.float32, kind="ExternalOutput").ap(),
    }

    with ExitStack() as ctx:
        tc = ctx.enter_context(tile.TileContext(nc))
        P = {}
        for pname, bufs, space in POOL_SPECS:
            P[pname] = ctx.enter_context(
                tc.tile_pool(name=pname, bufs=bufs, space=space))

        C = _setup_consts(nc, P, aps)
        st = {}
        import itertools
        filler = itertools.chain(
            _gen_a(nc, P, C, aps, 0, st),
            _gen_b_v(nc, P, C, 0, st),
            _gen_b_qk(nc, P, C, 0, st),
            _gen_a(nc, P, C, aps, 1, st),
            _gen_b_v(nc, P, C, 1, st),
            _gen_b_qk(nc, P, C, 1, st))
        _pull(filler, 10)
        c0 = _gen_c(nc, P, C, 0, st)
        i = 0
        for _ in c0:
            if i % 2 == 1:
                _pull(filler, 1)
            i += 1
        c1 = _gen_c(nc, P, C, 1, st)
        d0 = _gen_d(nc, P, C, aps, 0, st)
        i = 0
        for _ in c1:
            if i % 2 == 1:
                _pull(filler, 1)
            else:
                _pull(d0, 1)
            i += 1
        _exhaust(filler)
        _exhaust(d0)
        # batch 1 output projection: scores ring is idle by now — use it so
        # consecutive chunks double-buffer
        _exhaust(_gen_d(nc, P, C, aps, 1, st, pool="ps_sc"))

    nc.compile()
    return nc


_NC_CACHE = {}


def _get_nc():
    if "nc" not in _NC_CACHE:
        _NC_CACHE["nc"] = build()
    return _NC_CACHE["nc"]


def make_in_maps(x, Wqkv, bqkv, Wo, bo):
    bf = ml_dtypes.bfloat16
    x = np.asarray(x, dtype=np.float32)
    Wqkv16 = np.ascontiguousarray(np.asarray(Wqkv, np.float32).astype(bf))
    bqkv = np.asarray(bqkv, dtype=np.float32)
    Wo16 = np.ascontiguousarray(np.asarray(Wo, np.float32).astype(bf))
    bo = np.asarray(bo, dtype=np.float32)
    bqc = np.ascontiguousarray(bqkv.reshape(F3 // 128, 128).T)
    bqv = np.ascontiguousarray(bqkv[2 * D:].reshape(1, D))
    bo_r = np.ascontiguousarray(bo.reshape(1, D))
    x16 = np.ascontiguousarray(x.astype(bf).transpose(0, 2, 1))
    in_maps = []
    for c in range(N_CORES):
        in_maps.append({
            "xs": np.ascontiguousarray(x16[c * BPC:(c + 1) * BPC]),
            "wqkv": Wqkv16,
            "bqc": bqc,
            "bqv": bqv,
            "wo": Wo16,
            "bo": bo_r,
        })
    return in_maps


def run(x, Wqkv, bqkv, Wo, bo, trace=False, **kw):
    nc = _get_nc()
    in_maps = make_in_maps(x, Wqkv, bqkv, Wo, bo)
    res = run_bass_kernel_spmd(nc, in_maps, list(range(N_CORES)), trace=trace, **kw)
    out = np.concatenate([res.results[c]["ys"] for c in range(N_CORES)], axis=0)
    return out, res


def kernel(x, Wqkv, bqkv, Wo, bo):
    out, _ = run(x, Wqkv, bqkv, Wo, bo)
    return out


# revision 4
# speedup vs baseline: 1.0082x; 1.0082x over previous
"""Multi-head self-attention (B=16, N=784, D=768, H=12) on 8 trn2 cores.

Sharding: pure data-parallel over batch (2 batches per core, no collectives).
All matmuls bf16 with fp32 PSUM accumulation.

v2 over baseline: score matmuls for a head pair (which share one qkt tile —
head 2p at partitions 0:64, head 2p+1 at 64:128) are emitted back-to-back
into two separate PSUM tiles. They use disjoint PE row groups
(tile_position (0,0)/(64,0)) so the hardware runs them concurrently,
doubling effective PE-array utilization for the scores phase (K=64 would
otherwise use half the array). PV keeps the ones-augmented V slab (M=65,
softmax denominator in row 64); PV for the previous pair is fed from a work
queue, a few matmuls per tj slot, using one [65, 1024] PSUM tile at a time.
NOTE: col-tiled concurrent matmul pairs (two PE writers into one PSUM bank)
fault the device — do not re-introduce them.

PSUM budget (8 banks): scores ring 2 x [128,1024] (4 banks) + aux ring
1 x [128,1024] (2, projection accumulators, emitted atomically per chunk)
+ PV accumulator 1 x [65,1024] (2).

Loop order everywhere keeps the col-group innermost so consecutive MMs share
the stationary operand (saves the PE weight-swap drain bubble).
"""

from collections import deque
from contextlib import ExitStack

import ml_dtypes
import numpy as np

import concourse.mybir as mybir
import concourse.tile as tile
from concourse import bacc
from concourse.bass_utils import run_bass_kernel_spmd

dt = mybir.dt
AF = mybir.ActivationFunctionType

B, N, D = 16, 784, 768
H, HD = 12, 64
F3 = 3 * D  # 2304
N_CORES = 8
BPC = B // N_CORES  # batches per core

T_CHUNKS = [(i * 128, min(128, N - i * 128)) for i in range((N + 127) // 128)]
NT = len(T_CHUNKS)  # 7
ND = D // 128  # 6
COLS_N = [(0, 512), (512, N - 512)]
COLS_D = [(0, 512), (512, D - 512)]

BF = dt.bfloat16


def _setup_consts(nc, P, aps):
    bqc = P["konst"].tile([128, F3 // 128], dt.float32, name="bqc")
    nc.sync.dma_start(bqc[:], aps["bqc"][:])
    bqv_r = P["konst"].tile([1, D], dt.float32, name="bqv_r")
    nc.sync.dma_start(bqv_r[:], aps["bqv"][:])
    bqv_bc = P["konst"].tile([128, D], dt.float32, name="bqv_bc")
    nc.gpsimd.partition_broadcast(bqv_bc[:], bqv_r[0:1, :])
    bo_r = P["konst"].tile([1, D], dt.float32, name="bo_r")
    nc.sync.dma_start(bo_r[:], aps["bo"][:])
    bo_bc = P["konst"].tile([128, D], dt.float32, name="bo_bc")
    nc.gpsimd.partition_broadcast(bo_bc[:], bo_r[0:1, :])

    wq16, wo16 = [], []
    qmap = [nc.sync, nc.scalar]
    for di in range(ND):
        w = P["wq"].tile([128, F3], BF, name=f"wq{di}", tag="wq")
        qmap[di % 2].dma_start(w[:, 0:2 * D], aps["wqkv"][di * 128:(di + 1) * 128, 0:2 * D])
        wq16.append(w)
    for di in range(ND):
        w = wq16[di]
        qmap[di % 2].dma_start(w[:, 2 * D:F3], aps["wqkv"][di * 128:(di + 1) * 128, 2 * D:F3])
    for di in range(ND):
        w = P["wo"].tile([128, D], BF, name=f"wo{di}", tag="wo")
        nc.sync.dma_start(w[:], aps["wo"][di * 128:(di + 1) * 128, :])
        wo16.append(w)
    return dict(bqc=bqc, bqv_bc=bqv_bc, bo_bc=bo_bc, wq16=wq16, wo16=wo16)


def _gen_a(nc, P, C, aps, b, st):
    xt16 = [P["xt"].tile([128, N], BF, name=f"xt{b}_{di}", tag="xt")
            for di in range(ND)]
    for di in range(ND):
        nc.gpsimd.dma_start(xt16[di][:], aps["xs"][b, di * 128:(di + 1) * 128, :])
    yield
    st[f"xt{b}"] = xt16


def _gen_b_qk(nc, P, C, b, st):
    """Q,K transposed layout: 12 tiles [128, 784]. di outer / cg inner."""
    xt16 = st[f"xt{b}"]
    qkt16 = [None] * 12
    st[f"qkt{b}"] = qkt16
    for fi in [0, 6, 1, 7, 2, 8, 3, 9, 4, 10, 5, 11]:
        qk_ps = P["ps_aux"].tile([128, 1024], dt.float32, name="qk_ps", tag="aux")
        for di in range(ND):
            for (c0, cw) in reversed(COLS_N):
                nc.tensor.matmul(
                    qk_ps[:, c0:c0 + cw],
                    C["wq16"][di][:, fi * 128:(fi + 1) * 128],
                    xt16[di][:, c0:c0 + cw],
                    start=(di == 0), stop=(di == ND - 1))
        q = P["qkt"].tile([128, N], BF, name=f"qkt{b}_{fi}", tag="qkt")
        nc.vector.tensor_scalar_add(q[:], qk_ps[:, 0:N], C["bqc"][0:128, fi:fi + 1])
        qkt16[fi] = q
        yield


def _gen_b_v(nc, P, C, b, st):
    """V slab [t, 12, 65]: 65th column is 1.0 (softmax denominator trick)."""
    xt16 = st[f"xt{b}"]
    vt16 = []
    st[f"vt{b}"] = vt16
    for (t0, p), ti in zip(T_CHUNKS, range(NT)):
        v_ps = P["ps_aux"].tile([128, 1024], dt.float32, name="v_ps", tag="aux")
        for di in range(ND):
            for (c0, cw) in reversed(COLS_D):
                nc.tensor.matmul(
                    v_ps[0:p, c0:c0 + cw],
                    xt16[di][:, t0:t0 + p],
                    C["wq16"][di][:, 2 * D + c0:2 * D + c0 + cw],
                    start=(di == 0), stop=(di == ND - 1))
        vt = P["vt"].tile([128, H, HD + 1], BF, name=f"vt{b}_{ti}", tag="vt")
        nc.vector.tensor_tensor(
            vt[0:p, :, 0:HD],
            v_ps[0:p, 0:D].rearrange("p (h d) -> p h d", h=H),
            C["bqv_bc"][0:p, 0:D].rearrange("p (h d) -> p h d", h=H),
            mybir.AluOpType.add)
        nc.vector.memset(vt[0:p, :, HD:HD + 1], 1.0)
        vt16.append(vt)
        yield


def _pv_thunks(nc, P, b, st, p):
    """Work queue for pair p's PV + normalize (two heads, sequential).

    Per head: two PSUM accumulators (one per col group, 1 bank each), 14
    accumulating MMs tj-outer / cg-inner (consecutive MMs share the vt
    stationary; ex consumption is tj-monotone), then drain + normalize
    into ot16[p] rows hh*64. Row 64 of the accumulators is the softmax
    denominator (ones column of the V slab).
    """
    vt16, ex = st[f"vt{b}"], st[f"ex{b}_{p}"]
    work = deque()
    holder = {}

    def _alloc(hh):
        def f():
            holder[0] = P["ps_ot"].tile([HD + 1, 512], dt.float32,
                                        name=f"pv0_{p}_{hh}", tag="ot0")
            holder[1] = P["ps_ot"].tile([HD + 1, COLS_N[1][1]], dt.float32,
                                        name=f"pv1_{p}_{hh}", tag="ot1")
        return f

    def _mms(hh, tjs):
        h = 2 * p + hh

        def f():
            for tj in tjs:
                (t0, pj) = T_CHUNKS[tj]
                for ci, (c0, cw) in reversed(list(enumerate(COLS_N))):
                    nc.tensor.matmul(
                        holder[ci][0:HD + 1, 0:cw],
                        vt16[tj][0:pj, h, 0:HD + 1],
                        ex[2 * tj + hh][0:pj, c0:c0 + cw],
                        start=(tj == 0), stop=(tj == NT - 1))
        return f

    def _finish(hh):
        def f():
            if hh == 0:
                ot = P["ot"].tile([128, N], BF, name=f"ot{b}_{p}", tag="ot")
                st[f"ot{b}"][p] = ot
            ot = st[f"ot{b}"][p]
            osb = P["osb"].tile([HD + 1, N], dt.float32, name="osb65", tag="osb")
            for ci, (c0, cw) in enumerate(COLS_N):
                nc.vector.tensor_copy(osb[0:HD + 1, c0:c0 + cw],
                                      holder[ci][0:HD + 1, 0:cw])
            srow = P["recp"].tile([1, N], dt.float32, name="srow", tag="srow")
            nc.vector.tensor_copy(srow[0:1, :], osb[HD:HD + 1, :])
            rec = P["recp"].tile([1, N], dt.float32, name="rec", tag="rec")
            nc.vector.reciprocal_approx_fast(rec[0:1, :], srow[0:1, :])
            brec = P["brec"].tile([HD, N], dt.float32, name="brec", tag="brec")
            nc.gpsimd.partition_broadcast(brec[0:HD, :], rec[0:1, :])
            ro = hh * HD
            nc.vector.tensor_mul(ot[ro:ro + HD, :], osb[0:HD, :], brec[0:HD, :])
        return f

    for hh in range(2):
        work.append(_alloc(hh))
        work.append(_mms(hh, range(0, 2)))
        work.append(_mms(hh, range(2, 4)))
        work.append(_mms(hh, range(4, 6)))
        work.append(_mms(hh, range(6, NT)))
        work.append(_finish(hh))
    return work


def _pop(work, k):
    n = 0
    while work and n < k:
        work.popleft()()
        n += 1
    return n


def _gen_c(nc, P, C, b, st):
    """Attention for batch b. Yields once per (pair, tj)."""
    qkt16 = st[f"qkt{b}"]
    st[f"ot{b}"] = [None] * 6
    pv_work = deque()
    for p in range(6):
        ex = [P["ex"].tile([128, N], BF, name="ex", tag="ex") for _ in range(2 * NT)]
        st[f"ex{b}_{p}"] = ex
        if p >= 1:
            assert not pv_work
            pv_work = _pv_thunks(nc, P, b, st, p - 1)
        for (t0, pj), tj in zip(T_CHUNKS, range(NT)):
            # prev-pair PV (and the outer loop's filler, via yield) go in
            # front of the scores so the PE reaches the next score pair
            # after ACT has drained the previous one
            _pop(pv_work, 3)
            yield
            qt, kt = qkt16[p], qkt16[6 + p]
            sc_a = P["ps_sc"].tile([128, 1024], dt.float32, name="sc_a", tag="sc")
            sc_b = P["ps_sc"].tile([128, 1024], dt.float32, name="sc_b", tag="sc")
            for hh, sc in ((0, sc_a), (1, sc_b)):
                ro = hh * HD
                for (c0, cw) in reversed(COLS_N):
                    nc.tensor.matmul(
                        sc[0:pj, c0:c0 + cw],
                        kt[ro:ro + HD, t0:t0 + pj],
                        qt[ro:ro + HD, c0:c0 + cw],
                        start=True, stop=True)
            for hh, sc in ((0, sc_a), (1, sc_b)):
                nc.scalar.activation(ex[2 * tj + hh][0:pj, :], sc[0:pj, 0:N],
                                     AF.Exp, scale=float(HD) ** -0.5)
        _pop(pv_work, 99)  # finish prev pair's PV before ex tiles rotate far
    # epilogue: last pair's PV, yielding so the caller can interleave the
    # output projection's early chunks
    pv_work = _pv_thunks(nc, P, b, st, 5)
    while pv_work:
        _pop(pv_work, 3)
        yield


def _gen_d(nc, P, C, aps, b, st, pool="ps_aux"):
    """Output projection. On a 2-buf pool (the tail run), chunks are
    software-pipelined: each chunk's first four oi blocks run a chunk early
    so the late ot tiles (pairs 4,5 — normalized in the epilogue) never
    idle the PE."""
    ot16 = st[f"ot{b}"]
    pipelined = pool == "ps_sc"
    tag = "sc" if pool == "ps_sc" else "aux"
    tiles = [None] * NT

    def _mms(ti, oi_range, first_alloc):
        (t0, p) = T_CHUNKS[ti]
        if first_alloc:
            tiles[ti] = P[pool].tile([128, 1024], dt.float32, name="y_ps", tag=tag)
        y_ps = tiles[ti]
        for oi in oi_range:
            for (c0, cw) in reversed(COLS_D):
                nc.tensor.matmul(
                    y_ps[0:p, c0:c0 + cw],
                    ot16[oi][:, t0:t0 + p],
                    C["wo16"][oi][:, c0:c0 + cw],
                    start=(oi == 0), stop=(oi == ND - 1))

    def _drain(ti):
        (t0, p) = T_CHUNKS[ti]
        y32 = P["yout"].tile([128, D], dt.float32, name="y32", tag="y32")
        nc.vector.tensor_add(y32[0:p, :], tiles[ti][0:p, 0:D], C["bo_bc"][0:p, :])
        nc.gpsimd.dma_start(aps["ys"][b, t0:t0 + p, :], y32[0:p, :])

    if not pipelined:
        for ti in range(NT):
            _mms(ti, range(ND), True)
            yield
            _drain(ti)
            yield
    else:
        _mms(0, range(4), True)
        yield
        for ti in range(1, NT):
            _mms(ti, range(4), True)
            yield
            _mms(ti - 1, range(4, ND), False)
            _drain(ti - 1)
            yield
        _mms(NT - 1, range(4, ND), False)
        _drain(NT - 1)


def _exhaust(g):
    for _ in g:
        pass


def _pull(g, k):
    n = 0
    for _ in range(k):
        try:
            next(g)
        except StopIteration:
            break
        n += 1
    return n


POOL_SPECS = [
    ("konst", 1, "SBUF"), ("wq", ND, "SBUF"), ("wo", ND, "SBUF"),
    ("xt", 2 * ND, "SBUF"),
    ("qkt", 24, "SBUF"), ("vt", 2 * NT, "SBUF"), ("ex", 18, "SBUF"),
    ("ot", 12, "SBUF"), ("osb", 3, "SBUF"), ("recp", 2, "SBUF"),
    ("brec", 2, "SBUF"),
    ("yout", 2, "SBUF"),
    ("ps_sc", 2, "PSUM"), ("ps_aux", 1, "PSUM"), ("ps_ot", 1, "PSUM"),
]


def build():
    nc = bacc.Bacc("TRN2", target_bir_lowering=False, debug=False)

    aps = {
        "xs": nc.dram_tensor("xs", [BPC, D, N], BF, kind="ExternalInput").ap(),
        "wqkv": nc.dram_tensor("wqkv", [D, F3], BF, kind="ExternalInput").ap(),
        "bqc": nc.dram_tensor("bqc", [128, F3 // 128], dt.float32, kind="ExternalInput").ap(),
        "bqv": nc.dram_tensor("bqv", [1, D], dt.float32, kind="ExternalInput").ap(),
        "wo": nc.dram_tensor("wo", [D, D], BF, kind="ExternalInput").ap(),
        "bo": nc.dram_tensor("bo", [1, D], dt.float32, kind="ExternalInput").ap(),
        "ys": nc.dram_tensor("ys", [BPC, N, D], dt.float32, kind="ExternalOutput").ap(),
    }

    with ExitStack() as ctx:
        tc = ctx.enter_context(tile.TileContext(nc))
        P = {}
        for pname, bufs, space in POOL_SPECS:
            P[pname] = ctx.enter_context(
                tc.tile_pool(name=pname, bufs=bufs, space=space))

        C = _setup_consts(nc, P, aps)
        st = {}
        import itertools
        def _interleave(gq, gv):
            # qk pair chunks (2 per attention pair) woven with V chunks
            order = "qqvvvvvvvqqqqqqqqqq"  # qk{0,6}, v0-6, qk{1,7,2,8,...}
            for ch in order:
                g = gq if ch == "q" else gv
                try:
                    yield next(g)
                except StopIteration:
                    pass

        filler = itertools.chain(
            _gen_a(nc, P, C, aps, 0, st),
            _interleave(_gen_b_qk(nc, P, C, 0, st), _gen_b_v(nc, P, C, 0, st)),
            _gen_a(nc, P, C, aps, 1, st),
            _interleave(_gen_b_qk(nc, P, C, 1, st), _gen_b_v(nc, P, C, 1, st)))
        _pull(filler, 3)
        c0 = _gen_c(nc, P, C, 0, st)
        i = 0
        for _ in c0:
            if i == 7:
                _pull(filler, 2)  # both qkt tiles of the next pair
            elif i < 7 or i % 2 == 1:
                _pull(filler, 1)
            i += 1
        c1 = _gen_c(nc, P, C, 1, st)
        d0 = _gen_d(nc, P, C, aps, 0, st)
        d1 = _gen_d(nc, P, C, aps, 1, st, pool="ps_sc")
        i = 0
        for _ in c1:
            if i >= 42:
                pass
            elif i == 7:
                _pull(filler, 2)
            elif i < 7 or i % 2 == 1:
                _pull(filler, 1)
            else:
                _pull(d0, 1)
            i += 1
        _exhaust(filler)
        _exhaust(d0)
        _exhaust(d1)

    nc.compile()
    return nc


_NC_CACHE = {}


def _get_nc():
    if "nc" not in _NC_CACHE:
        _NC_CACHE["nc"] = build()
    return _NC_CACHE["nc"]


def make_in_maps(x, Wqkv, bqkv, Wo, bo):
    bf = ml_dtypes.bfloat16
    x = np.asarray(x, dtype=np.float32)
    Wqkv16 = np.ascontiguousarray(np.asarray(Wqkv, np.float32).astype(bf))
    bqkv = np.asarray(bqkv, dtype=np.float32)
    Wo16 = np.ascontiguousarray(np.asarray(Wo, np.float32).astype(bf))
    bo = np.asarray(bo, dtype=np.float32)
    bqc = np.ascontiguousarray(bqkv.reshape(F3 // 128, 128).T)
    bqv = np.ascontiguousarray(bqkv[2 * D:].reshape(1, D))
    bo_r = np.ascontiguousarray(bo.reshape(1, D))
    x16 = np.ascontiguousarray(x.astype(bf).transpose(0, 2, 1))
    in_maps = []
    for c in range(N_CORES):
        in_maps.append({
            "xs": np.ascontiguousarray(x16[c * BPC:(c + 1) * BPC]),
            "wqkv": Wqkv16,
            "bqc": bqc,
            "bqv": bqv,
            "wo": Wo16,
            "bo": bo_r,
        })
    return in_maps


def run(x, Wqkv, bqkv, Wo, bo, trace=False, **kw):
    nc = _get_nc()
    in_maps = make_in_maps(x, Wqkv, bqkv, Wo, bo)
    res = run_bass_kernel_spmd(nc, in_maps, list(range(N_CORES)), trace=trace, **kw)
    out = np.concatenate([res.results[c]["ys"] for c in range(N_CORES)], axis=0)
    return out, res


def kernel(x, Wqkv, bqkv, Wo, bo):
    out, _ = run(x, Wqkv, bqkv, Wo, bo)
    return out
